# revision 12
# baseline (speedup 1.0000x reference)
"""CrossAttention TRN2 kernel: 8-core (batch x head-group) sharded Bass/Tile implementation.

Reference computation (per batch b):
  q = x @ Wq; kv = k_in @ Wkv -> k, v   (H=16 heads, HD=64)
  attn = softmax(q k^T * HD^-0.5); attn = softmax(attn * attn_add); out = (attn @ v) @ Wproj + bproj

Sharding: core c -> batch b = c//2, heads h0 = (c%2)*8 .. +8. Each core computes a
partial (over its 8 heads) of out[b] in TRANSPOSED layout [c_out, n]; host sums the
two partials per batch, transposes back, and adds bias.

Device layout trick: host supplies x^T and k_in^T so all matmuls contract along
partitions without any on-device input transposes; softmax runs in natural [n, m]
layout (free-axis reductions, fused into the ACT exp pass via accum_out); the
attn^T needed by attn@v is built with PE transposes.
"""
import sys, os

sys.path.insert(0, "/opt/trn_rl_repo")

import numpy as np
import ml_dtypes

import concourse.bass as bass
import concourse.tile as tile
from concourse import bacc
import concourse.mybir as mybir
from concourse.bass_utils import run_bass_kernel_spmd
from concourse.masks import make_identity

B, N, C, H = 4, 1024, 1024, 16
HD = C // H          # 64
SCALE = HD ** -0.5   # 0.125
HPC = H // 2         # 8 heads per core
NT = N // 128        # 8 n-tiles
CT = C // 128        # 8 c-tiles
BF = mybir.dt.bfloat16
F32 = mybir.dt.float32
ALU = mybir.AluOpType
AF = mybir.ActivationFunctionType

_CACHE = {}


def _build():
    nc = bacc.Bacc("TRN2", target_bir_lowering=False, debug=False, num_devices=8)
    xT = nc.declare_dram_parameter("xT", [C, N], BF, isOutput=False)
    kT = nc.declare_dram_parameter("kT", [C, N], BF, isOutput=False)
    A = nc.declare_dram_parameter("A", [N, N], BF, isOutput=False)
    wq = nc.declare_dram_parameter("wq", [C, HPC * HD], BF, isOutput=False)
    wk = nc.declare_dram_parameter("wk", [C, HPC * HD], BF, isOutput=False)
    wv = nc.declare_dram_parameter("wv", [C, HPC * HD], BF, isOutput=False)
    wp = nc.declare_dram_parameter("wp", [HPC * HD, C], BF, isOutput=False)
    outT = nc.declare_dram_parameter("outT", [C, N], F32, isOutput=True)

    with tile.TileContext(nc) as tc:
        _emit(nc, tc, xT, kT, A, wq, wk, wv, wp, outT)
    nc.compile()
    return nc


def _emit(nc, tc, xT, kT, A, wq, wk, wv, wp, outT):
    from contextlib import ExitStack

    ctx = ExitStack()
    with ctx:
        persist = ctx.enter_context(tc.tile_pool(name="persist", bufs=1))
        ps_misc = ctx.enter_context(tc.tile_pool(name="ps_misc", bufs=2, space="PSUM"))

        ident = persist.tile([128, 128], BF)
        make_identity(nc, ident)

        a_sb = persist.tile([128, NT, N], BF)
        qTh = persist.tile([128, HPC // 2, N], BF)   # 2-head packs of q^T
        kTh = persist.tile([128, HPC // 2, N], BF)
        v_sb = persist.tile([128, NT, HPC * HD], BF)  # v in natural [m, hd] layout
        wp_sb = persist.tile([128, HPC * HD // 128, C], BF)

        # ---- inputs (k-side first: v-projection is needed earliest) ----
        phA = ctx.enter_context(tc.tile_pool(name="phA", bufs=1))
        wk_sb = phA.tile([128, CT, HPC * HD], BF)
        nc.sync.dma_start(out=wk_sb, in_=wk.rearrange("(t p) m -> p t m", p=128))
        kt = phA.tile([128, CT, N], BF)
        nc.sync.dma_start(out=kt, in_=kT.rearrange("(t p) n -> p t n", p=128))
        wv_sb = phA.tile([128, CT, HPC * HD], BF)
        nc.sync.dma_start(out=wv_sb, in_=wv.rearrange("(t p) m -> p t m", p=128))
        wq_sb = phA.tile([128, CT, HPC * HD], BF)
        nc.sync.dma_start(out=wq_sb, in_=wq.rearrange("(t p) m -> p t m", p=128))
        xt = phA.tile([128, CT, N], BF)
        nc.sync.dma_start(out=xt, in_=xT.rearrange("(t p) n -> p t n", p=128))
        # off the critical path: attention modulator + proj weights
        nc.gpsimd.dma_start(out=a_sb, in_=A.rearrange("(t p) m -> p t m", p=128))
        nc.gpsimd.dma_start(out=wp_sb, in_=wp.rearrange("(t p) m -> p t m", p=128))

        def qk_proj(p):
            for half in range(2):
                cols = bass.ts(half, 512)
                ps = ps_misc.tile([128, 512], F32, tag="m1")
                for ct in range(CT):
                    nc.tensor.matmul(
                        ps, wq_sb[:, ct, bass.ts(p, 128)], xt[:, ct, cols],
                        start=(ct == 0), stop=(ct == CT - 1))
                nc.vector.tensor_copy(qTh[:, p, cols], ps)
                ps2 = ps_misc.tile([128, 512], F32, tag="m1")
                for ct in range(CT):
                    nc.tensor.matmul(
                        ps2, wk_sb[:, ct, bass.ts(p, 128)], kt[:, ct, cols],
                        start=(ct == 0), stop=(ct == CT - 1))
                nc.vector.tensor_copy(kTh[:, p, cols], ps2)

        def v_proj():
            for mt in range(NT):
                ps = ps_misc.tile([128, 512], F32, tag="m1")
                for ct in range(CT):
                    nc.tensor.matmul(
                        ps, kt[:, ct, bass.ts(mt, 128)], wv_sb[:, ct, :],
                        start=(ct == 0), stop=(ct == CT - 1))
                nc.vector.tensor_copy(v_sb[:, mt, :], ps)

        # ---- Phase B: attention per head ----
        ps_s = ctx.enter_context(tc.tile_pool(name="ps_s", bufs=2, space="PSUM"))
        ps_o = ctx.enter_context(tc.tile_pool(name="ps_o", bufs=1, space="PSUM"))
        e1_pool = ctx.enter_context(tc.tile_pool(name="e1", bufs=4))
        t_pool = ctx.enter_context(tc.tile_pool(name="tt", bufs=4))
        e2_pool = ctx.enter_context(tc.tile_pool(name="e2", bufs=4))
        e2n_pool = ctx.enter_context(tc.tile_pool(name="e2n", bufs=2))
        atT_pool = ctx.enter_context(tc.tile_pool(name="atT", bufs=2))
        st_pool = ctx.enter_context(tc.tile_pool(name="st", bufs=24))
        oT_pool = ctx.enter_context(tc.tile_pool(name="oT", bufs=1))
        fin_pool = ctx.enter_context(tc.tile_pool(name="fin", bufs=3))

        oT = oT_pool.tile([128, HPC // 2, N], BF)
        pair_state = {}

        def chain_nt(h, e2n, nt):
            p, off = h // 2, (h % 2) * 64
            s = ps_s.tile([128, N], F32, tag="s", name=f"s{h}_{nt}")
            for mc in range(2):
                nc.tensor.matmul(
                    s[:, bass.ts(mc, 512)],
                    qTh[off:off + 64, p, bass.ts(nt, 128)],
                    kTh[off:off + 64, p, bass.ts(mc, 512)],
                    start=True, stop=True)
            e1 = e1_pool.tile([128, N], BF, tag="e1", name=f"e1_{h}_{nt}")
            r1 = st_pool.tile([128, 1], F32, tag="st", name=f"r1_{h}_{nt}")
            nc.scalar.activation(e1, s, AF.Exp, scale=SCALE, accum_out=r1)
            rc1 = st_pool.tile([128, 1], F32, tag="st", name=f"rc1_{h}_{nt}")
            nc.vector.reciprocal(rc1, r1)
            t = t_pool.tile([128, N], BF, tag="t", name=f"t{h}_{nt}")
            nc.vector.scalar_tensor_tensor(
                out=t, in0=e1, scalar=rc1, in1=a_sb[:, nt, :],
                op0=ALU.mult, op1=ALU.mult)
            e2 = e2_pool.tile([128, N], BF, tag="e2", name=f"e2_{h}_{nt}")
            r2 = st_pool.tile([128, 1], F32, tag="st", name=f"r2_{h}_{nt}")
            nc.scalar.activation(e2, t, AF.Exp, accum_out=r2)
            rc2 = st_pool.tile([128, 1], F32, tag="st", name=f"rc2_{h}_{nt}")
            nc.vector.reciprocal(rc2, r2)
            if nt % 2 == 0:
                nc.vector.tensor_scalar_mul(e2n[:, nt, :], e2, rc2)
            else:
                nc.gpsimd.tensor_scalar_mul(e2n[:, nt, :], e2, rc2)

        def tail_chunk(h, e2n, attnT, mt):
            p, off = h // 2, (h % 2) * 64
            for q in range(2):
                pt = ps_misc.tile([128, 512], BF, tag="m1", name=f"pt{h}_{mt}_{q}")
                for j in range(4):
                    nt = q * 4 + j
                    nc.tensor.transpose(
                        pt[:, bass.ts(j, 128)],
                        e2n[:, nt, bass.ts(mt, 128)], ident)
                nc.vector.tensor_copy(attnT[:, mt, bass.ts(q, 512)], pt)
            if mt == 0 and off == 0:
                pair_state[p] = ps_o.tile([128, N], F32, tag="o", name=f"pso{p}")
            ps_pair = pair_state[p]
            for mc in range(2):
                nc.tensor.matmul(
                    ps_pair[off:off + 64, bass.ts(mc, 512)],
                    v_sb[:, mt, bass.ts(h, 64)],
                    attnT[:, mt, bass.ts(mc, 512)],
                    start=(mt == 0), stop=(mt == NT - 1))
            if mt == NT - 1 and off == 64:
                nc.vector.tensor_copy(oT[:, p, :], ps_pair)

        kvar = os.environ.get("KVAR", "full")

        def full_chain(h):
            e2n = e2n_pool.tile([128, NT, N], BF, tag="e2n", name=f"e2n{h}")
            for nt in range(NT):
                chain_nt(h, e2n, nt)
            return e2n

        def full_tail(h, e2n):
            p, off = h // 2, (h % 2) * 64
            attnT = atT_pool.tile([128, NT, N], BF, tag="atT", name=f"atT{h}")
            for mt in range(NT):
                for q in range(2):
                    pt = ps_misc.tile([128, 512], BF, tag="m1", name=f"pt{h}_{mt}_{q}")
                    for j in range(4):
                        nt = q * 4 + j
                        nc.tensor.transpose(
                            pt[:, bass.ts(j, 128)],
                            e2n[:, nt, bass.ts(mt, 128)], ident)
                    nc.vector.tensor_copy(attnT[:, mt, bass.ts(q, 512)], pt)
            if off == 0:
                pair_state[p] = ps_o.tile([128, N], F32, tag="o", name=f"pso{p}")
            ps_pair = pair_state[p]
            for mc in range(2):
                for mt in range(NT):
                    nc.tensor.matmul(
                        ps_pair[off:off + 64, bass.ts(mc, 512)],
                        v_sb[:, mt, bass.ts(h, 64)],
                        attnT[:, mt, bass.ts(mc, 512)],
                        start=(mt == 0), stop=(mt == NT - 1))
            if off == 64:
                nc.vector.tensor_copy(oT[:, p, :], ps_pair)

        v_proj()
        qk_proj(0)
        prev = None
        for h in range(HPC):
            e2n = full_chain(h)
            if h in (0, 2, 4):
                qk_proj(h // 2 + 1)
            if prev is not None:
                full_tail(*prev)
            prev = (h, e2n)
        full_tail(*prev)

        # ---- Phase C: row-sliced projection, transposed output ----
        for co in range(CT):
            for half in range(2):
                cols = bass.ts(half, 512)
                ps = ps_misc.tile([128, 512], F32, tag="m1")
                for p in range(HPC // 2):
                    nc.tensor.matmul(
                        ps, wp_sb[:, p, bass.ts(co, 128)], oT[:, p, cols],
                        start=(p == 0), stop=(p == HPC // 2 - 1))
                f = fin_pool.tile([128, 512], F32, tag="f")
                nc.vector.tensor_copy(f, ps)
                nc.sync.dma_start(out=outT[co * 128:(co + 1) * 128, half * 512:(half + 1) * 512], in_=f)


def _prep(inputs):
    """Host-side shard prep: slice/transpose/cast per core."""
    x = np.asarray(inputs["x"], np.float32)
    k_in = np.asarray(inputs["k_in"], np.float32)
    attn_add = np.asarray(inputs["attn_add"], np.float32)
    Wq = np.asarray(inputs["Wq"], np.float32)
    Wkv = np.asarray(inputs["Wkv"], np.float32)
    Wproj = np.asarray(inputs["Wproj"], np.float32)
    bf = ml_dtypes.bfloat16
    in_maps = []
    for core in range(8):
        b, g = core // 2, core % 2
        h0 = g * HPC * HD  # column offset of this core's heads
        in_maps.append({
            "xT": np.ascontiguousarray(x[b].T).astype(bf),
            "kT": np.ascontiguousarray(k_in[b].T).astype(bf),
            "A": attn_add[b].astype(bf),
            "wq": np.ascontiguousarray(Wq[:, h0:h0 + HPC * HD]).astype(bf),
            "wk": np.ascontiguousarray(Wkv[:, h0:h0 + HPC * HD]).astype(bf),
            "wv": np.ascontiguousarray(Wkv[:, C + h0:C + h0 + HPC * HD]).astype(bf),
            "wp": np.ascontiguousarray(Wproj[h0:h0 + HPC * HD, :]).astype(bf),
        })
    return in_maps


def kernel(**inputs):
    if "nc" not in _CACHE:
        _CACHE["nc"] = _build()
    nc = _CACHE["nc"]
    in_maps = _prep(inputs)
    res = run_bass_kernel_spmd(nc, in_maps, core_ids=list(range(8)))
    bproj = np.asarray(inputs["bproj"], np.float32)
    out = np.empty((B, N, C), np.float32)
    for b in range(B):
        acc = res.results[2 * b]["outT"] + res.results[2 * b + 1]["outT"]
        out[b] = acc.T + bproj
    return out
